# revision 3
# baseline (speedup 1.0000x reference)
"""Farthest Point Sampling (FPS) Bass/TRN2 kernel, v2 (proven: 12.68ms).

Problem: pos [16, 16384, 3] f32 -> indices [16*2048] int32 (exact FPS,
start index 0, ratio 1/8), bit-exact trajectory vs the f32 reference.

Sharding: 16 clouds -> 8 NeuronCores, 2 clouds/core, no cross-core comm.
Cloud layout [128 part, 128 free], point n -> (n//128, n%128).

v2 design (v1 was DVE-bound: 72% busy, ~5.3us DVE work per step-pair):
  - GpSimd tensor_reduce(axis=C) computes the global max M [1,1] across
    partitions (replaces v1's PE-transpose + DVE reduce round trip).
  - PE ones-matmul broadcasts M to Mb [128,1] in PSUM; the gather stts
    read it as their per-partition scalar directly from PSUM.
  - The gather uses 3 fused scalar_tensor_tensor ops:
    out=(DIST==Mb)*(-chan), accum_out=row-sum. Channels are pre-negated
    -X,-Y,-Z; the single-hot mask makes the accumulated row value the
    exact bit pattern of -pos[n*] (replaces v1's mask + [128,512] mul +
    [128,512] reduce, which was half the DVE load).
  - PE all-ones matmul sums those partials over partitions AND
    broadcasts: bc3 [128,3] in PSUM; one ACT copy lands it in the
    per-step slice of a persistent bias buffer bcbuf [128, 3S] that the
    next step's Squares read as bias APs. bcbuf row 0 is DMA'd out once
    at the end; no per-step output op.
  - No index channel: the host recovers n* by hashing the exact f32
    coord triples (the masked gather is bit-exact).
Per-step chain: ACT copy+sq x3 -> DVE add,add,min,rowmax -> GP redC ->
PE Mb -> DVE stt x3 -> PE bc3 -> ACT copy. Clouds A/B run anti-phased.
"""

import numpy as np
from contextlib import ExitStack

import concourse.bass as bass
import concourse.mybir as mybir
from concourse.bass_utils import run_bass_kernel_spmd

AT = mybir.ActivationFunctionType
AL = mybir.AluOpType
AX = mybir.AxisListType
F32 = mybir.dt.float32

B, N, S = 16, 16384, 2048
N_CORES = 8
N_CLOUDS = 2  # per core
BIG = 1.0e10

_CACHE = {}


def _build_fps_kernel(S=S, n_clouds=N_CLOUDS):
    nc = bass.Bass(trn_type="TRN2", detect_race_conditions=False)
    mega_d = nc.dram_tensor("mega", [n_clouds, 128, 384], F32, kind="ExternalInput")
    megn_d = nc.dram_tensor("megn", [n_clouds, 128, 384], F32, kind="ExternalInput")
    ones_d = nc.dram_tensor("ones", [128, 128], F32, kind="ExternalInput")
    bias0_d = nc.dram_tensor("bias0", [n_clouds, 128, 3], F32, kind="ExternalInput")
    out_d = nc.dram_tensor("outrow", [n_clouds, 3 * S], F32, kind="ExternalOutput")

    es = ExitStack()
    counter = [0]

    def sb(shape):
        counter[0] += 1
        return es.enter_context(nc.sbuf_tensor(f"sb{counter[0]}", shape, F32))

    def ps(shape):
        counter[0] += 1
        return es.enter_context(nc.psum_tensor(f"ps{counter[0]}", shape, F32))

    scr_d = sb([128, 128])  # DVE stt scratch (shared; DVE is in-order)
    ones = sb([128, 128])   # all-ones; [0:1,:] used as onesrow

    cl = []
    for c in range(n_clouds):
        cl.append(dict(
            mega=sb([128, 384]),
            megn=sb([128, 384]),
            sqx=sb([128, 128]), sqy=sb([128, 128]), sqz=sb([128, 128]),
            t1=sb([128, 128]), dd=sb([128, 128]),
            dist=sb([128, 128]),
            rm=sb([128, 1]),
            msb=sb([1, 1]),
            rs3=sb([128, 3]),
            bcbuf=sb([128, 3 * S]),
            mb_ps=ps([128, 1]),
            bc_ps=ps([128, 3]),
        ))

    sem_act = es.enter_context(nc.semaphore())
    sem_dve = es.enter_context(nc.semaphore())
    sem_gp = es.enter_context(nc.semaphore())
    sem_pe = es.enter_context(nc.semaphore())

    sems = {"act": sem_act, "dve": sem_dve, "gp": sem_gp, "pe": sem_pe}
    engines = {"act": nc.scalar, "dve": nc.vector, "gp": nc.gpsimd, "pe": nc.tensor}
    count = {k: 0 for k in sems}
    waited = {(a, b): 0 for a in sems for b in sems}

    def emit(eng, instr, inc=1):
        instr.then_inc(sems[eng], inc)
        count[eng] += inc
        return count[eng]

    def wait(consumer, producer, tick):
        if tick is None or consumer == producer or tick <= 0:
            return
        if waited[(consumer, producer)] < tick:
            engines[consumer].wait_ge(sems[producer], tick)
            waited[(consumer, producer)] = tick

    # ---- prologue
    for c in range(n_clouds):
        emit("gp", nc.gpsimd.dma_start(cl[c]["mega"][:], mega_d[c]), 16)
        emit("gp", nc.gpsimd.dma_start(cl[c]["megn"][:], megn_d[c]), 16)
        emit("gp", nc.gpsimd.dma_start(cl[c]["bcbuf"][:, 0:3], bias0_d[c]), 16)
    emit("gp", nc.gpsimd.dma_start(ones[:], ones_d[:]), 16)
    dma0 = count["gp"]
    wait("dve", "gp", dma0)
    wait("act", "gp", dma0)
    wait("pe", "gp", dma0)
    for c in range(n_clouds):
        emit("dve", nc.vector.memset(cl[c]["dist"][:], BIG))

    ticks = [dict(cp=0, add1=0, add2=0, stt=0, bc=0, redc=0, mb=0)
             for _ in range(n_clouds)]

    def act_phase(c, s):
        # land bc3 PSUM -> bcbuf slice (bias for step s) via DVE (cross-
        # engine sems have write-complete semantics; an ACT copy followed
        # by an ACT Square races the Square's bias prefetch against the
        # copy's write drain), then squares of s on ACT.
        t, tk = cl[c], ticks[c]
        base = 3 * (s - 1)
        bb = t["bcbuf"]
        if s >= 2:
            wait("dve", "pe", tk["bc"])
            tk["cp"] = emit("dve", nc.vector.tensor_tensor(
                bb[:, base:base + 3], t["bc_ps"][:, 0:3], ones[:, 0:3], AL.bypass))
            wait("act", "dve", tk["cp"])
        wait("act", "dve", tk["add2"])      # sq bufs consumed by prev adds
        tk["sqx"] = emit("act", nc.scalar.activation(
            t["sqx"][:], t["mega"][:, 0:128], AT.Square,
            bias=bb[:, base:base + 1], scale=1.0))
        tk["sqy"] = emit("act", nc.scalar.activation(
            t["sqy"][:], t["mega"][:, 128:256], AT.Square,
            bias=bb[:, base + 1:base + 2], scale=1.0))
        tk["sqz"] = emit("act", nc.scalar.activation(
            t["sqz"][:], t["mega"][:, 256:384], AT.Square,
            bias=bb[:, base + 2:base + 3], scale=1.0))

    def dve_update(c, s):
        t, tk = cl[c], ticks[c]
        wait("dve", "act", tk["sqy"])
        tk["add1"] = emit("dve", nc.vector.tensor_tensor(
            t["t1"][:], t["sqx"][:], t["sqy"][:], AL.add))
        wait("dve", "act", tk["sqz"])
        tk["add2"] = emit("dve", nc.vector.tensor_tensor(
            t["dd"][:], t["t1"][:], t["sqz"][:], AL.add))
        tk["min"] = emit("dve", nc.vector.tensor_tensor(
            t["dist"][:], t["dist"][:], t["dd"][:], AL.min))
        wait("dve", "gp", tk["redc"])  # rm consumed by prev GP reduce
        tk["rmax"] = emit("dve", nc.vector.tensor_reduce(
            t["rm"][:, 0:1], t["dist"][:], axis=AX.X, op=AL.max))

    def gp_redmax(c, s):
        # cross-partition max of the per-row maxima in one gpsimd op
        t, tk = cl[c], ticks[c]
        wait("gp", "dve", tk["rmax"])
        wait("gp", "pe", tk["mb"])  # msb consumed by prev PE broadcast
        tk["redc"] = emit("gp", nc.gpsimd.tensor_reduce(
            t["msb"][0:1, 0:1], t["rm"][:, 0:1], axis=AX.C, op=AL.max))

    def pe_mb(c, s):
        t, tk = cl[c], ticks[c]
        wait("pe", "gp", tk["redc"])
        wait("pe", "dve", tk["stt"])  # Mb PSUM consumed by prev stts
        tk["mb"] = emit("pe", nc.tensor.matmul(
            t["mb_ps"][:], ones[0:1, :], t["msb"][0:1, 0:1], start=True, stop=True))

    def dve_gathers(c, s):
        t, tk = cl[c], ticks[c]
        wait("dve", "pe", tk["mb"])
        for j in range(3):
            tk["stt"] = emit("dve", nc.vector.scalar_tensor_tensor(
                out=scr_d[:], in0=t["dist"][:], scalar=t["mb_ps"][:, 0:1],
                in1=t["megn"][:, 128 * j:128 * (j + 1)],
                op0=AL.is_equal, op1=AL.mult,
                accum_out=t["rs3"][:, j:j + 1]))

    def pe_bc(c, s):
        t, tk = cl[c], ticks[c]
        # bc_ps WAR vs the previous DVE copy is subsumed: stt(s) > cp(s)
        # in DVE program order.
        wait("pe", "dve", tk["stt"])
        tk["bc"] = emit("pe", nc.tensor.matmul(
            t["bc_ps"][:], ones[:], t["rs3"][:, 0:3], start=True, stop=True))

    # ---- steady state: A leads, B trails anti-phased
    A, Bc = 0, 1
    for s in range(1, S):
        act_phase(A, s)
        dve_update(A, s)
        gp_redmax(A, s)
        pe_mb(A, s)
        act_phase(Bc, s)
        dve_gathers(A, s)
        pe_bc(A, s)
        dve_update(Bc, s)
        gp_redmax(Bc, s)
        pe_mb(Bc, s)
        dve_gathers(Bc, s)
        pe_bc(Bc, s)

    # ---- epilogue: land the final bc3 (coords of the last pick), DMA out
    for c in range(n_clouds):
        t, tk = cl[c], ticks[c]
        wait("act", "pe", tk["bc"])
        tk["cpf"] = emit("act", nc.scalar.copy(
            t["bcbuf"][:, 3 * (S - 1):3 * (S - 1) + 3], t["bc_ps"][:, 0:3]))
        wait("gp", "act", tk["cpf"])
        emit("gp", nc.gpsimd.dma_start(out_d[c], t["bcbuf"][0:1, :]), 16)

    es.close()
    return nc


def _make_inputs(pos_pair):
    ncl = pos_pair.shape[0]
    mega = np.empty((ncl, 128, 384), np.float32)
    megn = np.empty((ncl, 128, 384), np.float32)
    bias0 = np.empty((ncl, 128, 3), np.float32)
    for c in range(ncl):
        for j in range(3):
            mega[c, :, j * 128:(j + 1) * 128] = pos_pair[c, :, j].reshape(128, 128)
        megn[c] = -mega[c]
        bias0[c, :, 0:3] = -pos_pair[c, 0]
    return {"mega": mega, "megn": megn, "bias0": bias0,
            "ones": np.ones((128, 128), np.float32)}


def _get_nc():
    if "nc" not in _CACHE:
        _CACHE["nc"] = _build_fps_kernel()
    return _CACHE["nc"]


def _decode_indices(outrow, pos_cloud, n_samp=S):
    """outrow [3S] f32 of negated coords -> local indices [S] via exact
    f32 coordinate hashing against pos_cloud [N, 3]."""
    key = np.ascontiguousarray(pos_cloud, dtype=np.float32)
    lut = {key[n].tobytes(): n for n in range(key.shape[0])}
    assert len(lut) == key.shape[0], "duplicate points break coord decoding"
    v = (-outrow.reshape(n_samp, 3)).astype(np.float32)
    loc = np.empty(n_samp, np.int32)
    loc[0] = 0
    for s in range(1, n_samp):
        loc[s] = lut[v[s].tobytes()]
    return loc


def run_on_cores(pos, **spmd_kwargs):
    """pos [16, 16384, 3] f32 -> (idx [16*2048] int32, BassKernelResults)."""
    pos = np.ascontiguousarray(np.asarray(pos, dtype=np.float32))
    assert pos.shape == (B, N, 3)
    nc = _get_nc()
    in_maps = [_make_inputs(pos[N_CLOUDS * c: N_CLOUDS * (c + 1)]) for c in range(N_CORES)]
    res = run_bass_kernel_spmd(nc, in_maps, core_ids=list(range(N_CORES)), **spmd_kwargs)
    idx = np.empty((B, S), np.int32)
    for core in range(N_CORES):
        outrow = res.results[core]["outrow"]  # [n_clouds, 3S]
        for c in range(N_CLOUDS):
            b = N_CLOUDS * core + c
            idx[b] = _decode_indices(outrow[c], pos[b]) + b * N
    return idx.reshape(-1), res


def kernel(pos):
    idx, _ = run_on_cores(pos)
    return idx


# revision 4
# speedup vs baseline: 1.1882x; 1.1882x over previous
"""Farthest Point Sampling (FPS) Bass/TRN2 kernel, v2 (proven: 12.68ms).

Problem: pos [16, 16384, 3] f32 -> indices [16*2048] int32 (exact FPS,
start index 0, ratio 1/8), bit-exact trajectory vs the f32 reference.

Sharding: 16 clouds -> 8 NeuronCores, 2 clouds/core, no cross-core comm.
Cloud layout [128 part, 128 free], point n -> (n//128, n%128).

v2 design (v1 was DVE-bound: 72% busy, ~5.3us DVE work per step-pair):
  - GpSimd tensor_reduce(axis=C) computes the global max M [1,1] across
    partitions (replaces v1's PE-transpose + DVE reduce round trip).
  - PE ones-matmul broadcasts M to Mb [128,1] in PSUM; the gather stts
    read it as their per-partition scalar directly from PSUM.
  - The gather uses 3 fused scalar_tensor_tensor ops:
    out=(DIST==Mb)*(-chan), accum_out=row-sum. Channels are pre-negated
    -X,-Y,-Z; the single-hot mask makes the accumulated row value the
    exact bit pattern of -pos[n*] (replaces v1's mask + [128,512] mul +
    [128,512] reduce, which was half the DVE load).
  - PE all-ones matmul sums those partials over partitions AND
    broadcasts: bc3 [128,3] in PSUM; one ACT copy lands it in the
    per-step slice of a persistent bias buffer bcbuf [128, 3S] that the
    next step's Squares read as bias APs. bcbuf row 0 is DMA'd out once
    at the end; no per-step output op.
  - No index channel: the host recovers n* by hashing the exact f32
    coord triples (the masked gather is bit-exact).
Per-step chain: ACT copy+sq x3 -> DVE add,add,min,rowmax -> GP redC ->
PE Mb -> DVE stt x3 -> PE bc3 -> ACT copy. Clouds A/B run anti-phased.
"""

import numpy as np
from contextlib import ExitStack

import concourse.bass as bass
import concourse.mybir as mybir
from concourse.bass_utils import run_bass_kernel_spmd

AT = mybir.ActivationFunctionType
AL = mybir.AluOpType
AX = mybir.AxisListType
F32 = mybir.dt.float32

B, N, S = 16, 16384, 2048
N_CORES = 8
N_CLOUDS = 2  # per core
BIG = 1.0e10

_CACHE = {}


def _build_fps_kernel(S=S, n_clouds=N_CLOUDS):
    nc = bass.Bass(trn_type="TRN2", detect_race_conditions=False)
    mega_d = nc.dram_tensor("mega", [n_clouds, 128, 384], F32, kind="ExternalInput")
    megn_d = nc.dram_tensor("megn", [n_clouds, 128, 384], F32, kind="ExternalInput")
    ones_d = nc.dram_tensor("ones", [128, 128], F32, kind="ExternalInput")
    bias0_d = nc.dram_tensor("bias0", [n_clouds, 128, 3], F32, kind="ExternalInput")
    out_d = nc.dram_tensor("outrow", [n_clouds, 3 * S], F32, kind="ExternalOutput")

    es = ExitStack()
    counter = [0]

    def sb(shape):
        counter[0] += 1
        return es.enter_context(nc.sbuf_tensor(f"sb{counter[0]}", shape, F32))

    def ps(shape):
        counter[0] += 1
        return es.enter_context(nc.psum_tensor(f"ps{counter[0]}", shape, F32))

    scr_d = sb([128, 128])  # DVE stt scratch (shared; DVE is in-order)
    ones = sb([128, 128])   # all-ones; [0:1,:] used as onesrow

    cl = []
    for c in range(n_clouds):
        cl.append(dict(
            mega=sb([128, 384]),
            megn=sb([128, 384]),
            sqx=sb([128, 128]), sqy=sb([128, 128]), sqz=sb([128, 128]),
            t1=sb([128, 128]), dd=sb([128, 128]),
            dist=sb([128, 128]),
            rm=sb([128, 1]),
            msb=sb([1, 1]),
            rs3=sb([128, 3]),
            bcbuf=sb([128, 3 * S]),
            mb_ps=ps([128, 1]),
            bc_ps=ps([128, 3]),
        ))

    sem_act = es.enter_context(nc.semaphore())
    sem_dve = es.enter_context(nc.semaphore())
    sem_gp = es.enter_context(nc.semaphore())
    sem_pe = es.enter_context(nc.semaphore())

    sems = {"act": sem_act, "dve": sem_dve, "gp": sem_gp, "pe": sem_pe}
    engines = {"act": nc.scalar, "dve": nc.vector, "gp": nc.gpsimd, "pe": nc.tensor}
    count = {k: 0 for k in sems}
    waited = {(a, b): 0 for a in sems for b in sems}

    def emit(eng, instr, inc=1):
        instr.then_inc(sems[eng], inc)
        count[eng] += inc
        return count[eng]

    def wait(consumer, producer, tick):
        if tick is None or consumer == producer or tick <= 0:
            return
        if waited[(consumer, producer)] < tick:
            engines[consumer].wait_ge(sems[producer], tick)
            waited[(consumer, producer)] = tick

    # ---- prologue
    for c in range(n_clouds):
        emit("gp", nc.gpsimd.dma_start(cl[c]["mega"][:], mega_d[c]), 16)
        emit("gp", nc.gpsimd.dma_start(cl[c]["megn"][:], megn_d[c]), 16)
        emit("gp", nc.gpsimd.dma_start(cl[c]["bcbuf"][:, 0:3], bias0_d[c]), 16)
    emit("gp", nc.gpsimd.dma_start(ones[:], ones_d[:]), 16)
    dma0 = count["gp"]
    wait("dve", "gp", dma0)
    wait("act", "gp", dma0)
    wait("pe", "gp", dma0)
    for c in range(n_clouds):
        emit("dve", nc.vector.memset(cl[c]["dist"][:], BIG))

    ticks = [dict(cp=0, add1=0, add2=0, stt=0, bc=0, redc=0, mb=0)
             for _ in range(n_clouds)]

    def act_phase(c, s):
        # land bc3 PSUM -> bcbuf slice (bias for step s) via DVE (cross-
        # engine sems have write-complete semantics; an ACT copy followed
        # by an ACT Square races the Square's bias prefetch against the
        # copy's write drain), then squares of s on ACT.
        t, tk = cl[c], ticks[c]
        base = 3 * (s - 1)
        bb = t["bcbuf"]
        if s >= 2:
            wait("dve", "pe", tk["bc"])
            tk["cp"] = emit("dve", nc.vector.tensor_tensor(
                bb[:, base:base + 3], t["bc_ps"][:, 0:3], ones[:, 0:3], AL.bypass))
            wait("act", "dve", tk["cp"])
        wait("act", "dve", tk["add2"])      # sq bufs consumed by prev adds
        tk["sqx"] = emit("act", nc.scalar.activation(
            t["sqx"][:], t["mega"][:, 0:128], AT.Square,
            bias=bb[:, base:base + 1], scale=1.0))
        tk["sqy"] = emit("act", nc.scalar.activation(
            t["sqy"][:], t["mega"][:, 128:256], AT.Square,
            bias=bb[:, base + 1:base + 2], scale=1.0))
        tk["sqz"] = emit("act", nc.scalar.activation(
            t["sqz"][:], t["mega"][:, 256:384], AT.Square,
            bias=bb[:, base + 2:base + 3], scale=1.0))

    def dve_update(c, s):
        t, tk = cl[c], ticks[c]
        wait("dve", "act", tk["sqy"])
        tk["add1"] = emit("dve", nc.vector.tensor_tensor(
            t["t1"][:], t["sqx"][:], t["sqy"][:], AL.add))
        wait("dve", "act", tk["sqz"])
        tk["add2"] = emit("dve", nc.vector.tensor_tensor(
            t["dd"][:], t["t1"][:], t["sqz"][:], AL.add))
        tk["min"] = emit("dve", nc.vector.tensor_tensor(
            t["dist"][:], t["dist"][:], t["dd"][:], AL.min))
        wait("dve", "gp", tk["redc"])  # rm consumed by prev GP reduce
        tk["rmax"] = emit("dve", nc.vector.tensor_reduce(
            t["rm"][:, 0:1], t["dist"][:], axis=AX.X, op=AL.max))

    def gp_redmax(c, s):
        # cross-partition max of the per-row maxima in one gpsimd op
        t, tk = cl[c], ticks[c]
        wait("gp", "dve", tk["rmax"])
        wait("gp", "pe", tk["mb"])  # msb consumed by prev PE broadcast
        tk["redc"] = emit("gp", nc.gpsimd.tensor_reduce(
            t["msb"][0:1, 0:1], t["rm"][:, 0:1], axis=AX.C, op=AL.max))

    def pe_mb(c, s):
        t, tk = cl[c], ticks[c]
        wait("pe", "gp", tk["redc"])
        wait("pe", "dve", tk["stt"])  # Mb PSUM consumed by prev stts
        tk["mb"] = emit("pe", nc.tensor.matmul(
            t["mb_ps"][:], ones[0:1, :], t["msb"][0:1, 0:1], start=True, stop=True))

    def dve_gathers(c, s):
        t, tk = cl[c], ticks[c]
        wait("dve", "pe", tk["mb"])
        for j in range(3):
            tk["stt"] = emit("dve", nc.vector.scalar_tensor_tensor(
                out=scr_d[:], in0=t["dist"][:], scalar=t["mb_ps"][:, 0:1],
                in1=t["megn"][:, 128 * j:128 * (j + 1)],
                op0=AL.is_equal, op1=AL.mult,
                accum_out=t["rs3"][:, j:j + 1]))

    def pe_bc(c, s):
        t, tk = cl[c], ticks[c]
        # bc_ps WAR vs the previous DVE copy is subsumed: stt(s) > cp(s)
        # in DVE program order.
        wait("pe", "dve", tk["stt"])
        tk["bc"] = emit("pe", nc.tensor.matmul(
            t["bc_ps"][:], ones[:], t["rs3"][:, 0:3], start=True, stop=True))

    # ---- steady state: A leads, B trails anti-phased
    A, Bc = 0, 1
    for s in range(1, S):
        act_phase(A, s)
        dve_update(A, s)
        gp_redmax(A, s)
        pe_mb(A, s)
        dve_gathers(A, s)
        pe_bc(A, s)
        act_phase(Bc, s)
        dve_update(Bc, s)
        gp_redmax(Bc, s)
        pe_mb(Bc, s)
        dve_gathers(Bc, s)
        pe_bc(Bc, s)

    # ---- epilogue: land the final bc3 (coords of the last pick), DMA out
    for c in range(n_clouds):
        t, tk = cl[c], ticks[c]
        wait("act", "pe", tk["bc"])
        tk["cpf"] = emit("act", nc.scalar.copy(
            t["bcbuf"][:, 3 * (S - 1):3 * (S - 1) + 3], t["bc_ps"][:, 0:3]))
        wait("gp", "act", tk["cpf"])
        emit("gp", nc.gpsimd.dma_start(out_d[c], t["bcbuf"][0:1, :]), 16)

    es.close()
    return nc


def _make_inputs(pos_pair):
    ncl = pos_pair.shape[0]
    mega = np.empty((ncl, 128, 384), np.float32)
    megn = np.empty((ncl, 128, 384), np.float32)
    bias0 = np.empty((ncl, 128, 3), np.float32)
    for c in range(ncl):
        for j in range(3):
            mega[c, :, j * 128:(j + 1) * 128] = pos_pair[c, :, j].reshape(128, 128)
        megn[c] = -mega[c]
        bias0[c, :, 0:3] = -pos_pair[c, 0]
    return {"mega": mega, "megn": megn, "bias0": bias0,
            "ones": np.ones((128, 128), np.float32)}


def _get_nc():
    if "nc" not in _CACHE:
        _CACHE["nc"] = _build_fps_kernel()
    return _CACHE["nc"]


def _decode_indices(outrow, pos_cloud, n_samp=S):
    """outrow [3S] f32 of negated coords -> local indices [S] via exact
    f32 coordinate hashing against pos_cloud [N, 3]."""
    key = np.ascontiguousarray(pos_cloud, dtype=np.float32)
    lut = {key[n].tobytes(): n for n in range(key.shape[0])}
    assert len(lut) == key.shape[0], "duplicate points break coord decoding"
    v = (-outrow.reshape(n_samp, 3)).astype(np.float32)
    loc = np.empty(n_samp, np.int32)
    loc[0] = 0
    for s in range(1, n_samp):
        loc[s] = lut[v[s].tobytes()]
    return loc


def run_on_cores(pos, **spmd_kwargs):
    """pos [16, 16384, 3] f32 -> (idx [16*2048] int32, BassKernelResults)."""
    pos = np.ascontiguousarray(np.asarray(pos, dtype=np.float32))
    assert pos.shape == (B, N, 3)
    nc = _get_nc()
    in_maps = [_make_inputs(pos[N_CLOUDS * c: N_CLOUDS * (c + 1)]) for c in range(N_CORES)]
    res = run_bass_kernel_spmd(nc, in_maps, core_ids=list(range(N_CORES)), **spmd_kwargs)
    idx = np.empty((B, S), np.int32)
    for core in range(N_CORES):
        outrow = res.results[core]["outrow"]  # [n_clouds, 3S]
        for c in range(N_CLOUDS):
            b = N_CLOUDS * core + c
            idx[b] = _decode_indices(outrow[c], pos[b]) + b * N
    return idx.reshape(-1), res


def kernel(pos):
    idx, _ = run_on_cores(pos)
    return idx
